# revision 17
# baseline (speedup 1.0000x reference)
"""Fused transformer block (GN -> causal MHA -> GN -> MLP, residuals) on
8 Trainium2 NeuronCores.

Sharding: 8-way tensor parallel over the 16 attention heads (2 heads per
core, both batch elements on every core); everything after attention is
sequence(T)-sharded (each core owns a 512-column slice of the flattened
[B*T] axis). Cross-core traffic: one tiny stats AllGather per GroupNorm
(partial sums of per-(batch,group) statistics) and one 2MB AllToAll that
re-shards attention outputs from head-sharded to T-sharded. Weights for
the T-sharded matmuls (w_aproj/w_fc/w_mproj) are streamed from HBM.

All matmuls run as fp32r (full-rate fp32 with PE-internal mantissa
rounding); GroupNorm scale/shift is folded into the QKV weights so the
16MB activation tensor is read straight into the tensor engine.
"""

import os
import sys

sys.path.insert(0, "/opt/trn_rl_repo")

import numpy as np

import concourse.bass as bass
import concourse.bacc as bacc
import concourse.mybir as mybir
import concourse.tile as tile
from concourse.alu_op_type import AluOpType

f32 = mybir.dt.float32
f32r = mybir.dt.float32r
AF = mybir.ActivationFunctionType
AX = mybir.AxisListType

NC = 8          # cores
B = 2
H = 1024
NH = 16
HD = 64
NG = 32         # groupnorm groups
H4 = 4 * H
EPS = 1e-05
HPC = NH // NC  # heads per core = 2
KCH = H // 128  # 8 input-channel chunks


def build(T_b, sim_safe_gelu=False):
    """Build + bacc-compile the SPMD kernel for per-batch seq len T_b."""
    gelu_fn = AF.Sigmoid if sim_safe_gelu else AF.Gelu_apprx_tanh
    QT = T_b // 4          # q-tile width == per-core T slice
    SLICE = B * T_b // NC  # columns of the flattened B*T axis per core
    assert SLICE == QT
    NKT = T_b // 128       # k-tiles per batch
    THL = T_b // 2         # half-batch (per qkv sweep)
    DI = QT // 128         # diagonal straddle patterns
    NSTAT = T_b * (H // NG)
    RG = [list(range(NC))]

    nc = bacc.Bacc("TRN2", target_bir_lowering=False, debug=False,
                   num_devices=NC)

    # ---- DRAM I/O ----
    d_xT = nc.dram_tensor("xT", [H, B * T_b], f32r, kind="ExternalInput")
    d_xres = nc.dram_tensor("x_res", [H, SLICE], f32, kind="ExternalInput")
    d_wqk = nc.dram_tensor("wqk", [H, 256], f32r, kind="ExternalInput")
    d_wkv = nc.dram_tensor("wkv", [H, 256], f32r, kind="ExternalInput")
    d_bqk = nc.dram_tensor("bqk", [2, 256], f32r, kind="ExternalInput")
    d_bkv = nc.dram_tensor("bkv", [2, 256], f32r, kind="ExternalInput")
    d_id2 = nc.dram_tensor("id2", [2, 2], f32r, kind="ExternalInput")
    d_wap = nc.dram_tensor("w_aproj", [H, H], f32r, kind="ExternalInput")
    d_bap = nc.dram_tensor("b_aproj", [H, 1], f32, kind="ExternalInput")
    d_wfc = nc.dram_tensor("w_fc", [H, H4], f32r, kind="ExternalInput")
    d_bfc = nc.dram_tensor("b_fc", [H4, 1], f32, kind="ExternalInput")
    d_wmp = nc.dram_tensor("w_mproj", [H4, H], f32r, kind="ExternalInput")
    d_bmp = nc.dram_tensor("b_mproj", [H, 1], f32, kind="ExternalInput")
    d_ln1w = nc.dram_tensor("ln1w", [H, 1], f32, kind="ExternalInput")
    d_ln1b = nc.dram_tensor("ln1b", [H, 1], f32, kind="ExternalInput")
    d_ln2w = nc.dram_tensor("ln2w", [H, 1], f32, kind="ExternalInput")
    d_ln2b = nc.dram_tensor("ln2b", [H, 1], f32, kind="ExternalInput")
    d_gm = nc.dram_tensor("gmask", [KCH, 128, NG], f32r, kind="ExternalInput")
    d_gmT = nc.dram_tensor("gmaskT", [KCH, NG, 128], f32r, kind="ExternalInput")
    d_mb0 = nc.dram_tensor("Mb0", [2, 128, NG], f32r, kind="ExternalInput")
    d_mb1 = nc.dram_tensor("Mb1", [2, 128, NG], f32r, kind="ExternalInput")
    d_mmy = nc.dram_tensor("Mmy", [2, 128, NG], f32r, kind="ExternalInput")
    d_hsel = nc.dram_tensor("hsel", [KCH, 16, 128], f32r, kind="ExternalInput")
    d_cmask = nc.dram_tensor("cmaskA", [DI, 128, QT], f32, kind="ExternalInput")
    d_ones = nc.dram_tensor("ones_c", [128, QT], f32r, kind="ExternalInput")

    d_out = nc.dram_tensor("x2T", [H, SLICE], f32, kind="ExternalOutput")
    d_pres = nc.dram_tensor("present", [2, B, HPC, T_b, HD], f32,
                            kind="ExternalOutput")

    with tile.TileContext(nc) as tc:
        with (
            tc.tile_pool(name="ctp", bufs=1) as ctp,     # constants
            tc.tile_pool(name="vecp", bufs=1) as vecp,   # [128,1] vectors
            tc.tile_pool(name="xrp", bufs=1) as xrp,     # x residual slice
            tc.tile_pool(name="x1p", bufs=1) as x1p,     # x1 slice
            tc.tile_pool(name="smp", bufs=1) as smp,     # small stat tiles
            tc.tile_pool(name="ps", bufs=4, space="PSUM") as ps,
            tc.tile_pool(name="dram", bufs=1, space="DRAM") as dram,
        ):
            # ---------- constants ----------
            ones_c = ctp.tile([128, QT], f32r, tag="ones_c")
            nc.sync.dma_start(ones_c[:], d_ones[:])
            cm_sb = []
            for di in range(DI):
                t = ctp.tile([128, QT], f32, tag=f"cm{di}", name=f"cm{di}")
                nc.sync.dma_start(t[:], d_cmask[di])
                cm_sb.append(t)
            gm_sb, gmT_sb = [], []
            for k in range(KCH):
                t = ctp.tile([128, NG], f32r, tag=f"gm{k}", name=f"gm{k}")
                nc.sync.dma_start(t[:], d_gm[k])
                gm_sb.append(t)
                t2 = ctp.tile([NG, 128], f32r, tag=f"gmT{k}", name=f"gmT{k}")
                nc.sync.dma_start(t2[:], d_gmT[k])
                gmT_sb.append(t2)
            mb_sb = {}
            for nm, dt_ in (("b0", d_mb0), ("b1", d_mb1), ("my", d_mmy)):
                mb_sb[nm] = []
                for c2 in range(2):
                    t = ctp.tile([128, NG], f32r, tag=f"M{nm}{c2}",
                                 name=f"M{nm}{c2}")
                    nc.sync.dma_start(t[:], dt_[c2])
                    mb_sb[nm].append(t)

            def vec_chunks(dten, n, tag):
                out = []
                for k in range(n):
                    t = vecp.tile([128, 1], dten.dtype, tag=f"{tag}{k}",
                                  name=f"{tag}{k}")
                    nc.sync.dma_start(t[:], dten[128 * k:128 * (k + 1), :])
                    out.append(t)
                return out

            ln1w_sb = vec_chunks(d_ln1w, KCH, "ln1w")
            ln1b_sb = vec_chunks(d_ln1b, KCH, "ln1b")
            ln2w_sb = vec_chunks(d_ln2w, KCH, "ln2w")
            ln2b_sb = vec_chunks(d_ln2b, KCH, "ln2b")
            bap_sb = vec_chunks(d_bap, KCH, "bap")
            bfc_sb = vec_chunks(d_bfc, H4 // 128, "bfc")
            bmp_sb = vec_chunks(d_bmp, KCH, "bmp")
            bqk_sb = ctp.tile([2, 256], f32r, tag="bqk")
            nc.sync.dma_start(bqk_sb[:], d_bqk[:])
            bkv_sb = ctp.tile([2, 256], f32r, tag="bkv")
            nc.sync.dma_start(bkv_sb[:], d_bkv[:])
            id2_sb = ctp.tile([2, 2], f32r, tag="id2")
            nc.sync.dma_start(id2_sb[:], d_id2[:])
            eps_sb = ctp.tile([NG, 1], f32, tag="eps")
            nc.vector.memset(eps_sb[:], EPS)

            x_res = []
            for k in range(KCH):
                t = xrp.tile([128, SLICE], f32, tag=f"xres{k}",
                             name=f"xres{k}")
                nc.sync.dma_start(t[:], d_xres[128 * k:128 * (k + 1), :])
                x_res.append(t)

            # ---------- GroupNorm helpers ----------
            def gn_partials(src_chunks, pool, tag):
                """[NG,2] (sum, sumsq) partials over this core's slice."""
                psg = ps.tile([NG, 2], f32, tag="psB", bufs=4)
                for k in range(KCH):
                    part = pool.tile([128, 2], f32r, tag=f"part{tag}", bufs=2,
                                     name=f"part{tag}{k}")
                    scr = pool.tile([128, SLICE], f32, tag=f"scr{tag}", bufs=2,
                                    name=f"scr{tag}{k}")
                    with nc.allow_low_precision(reason="f32r is 4B fp32"):
                        nc.vector.reduce_sum(part[:, 0:1], src_chunks[k][:],
                                             axis=AX.X)
                        nc.scalar.activation(scr[:], src_chunks[k][:],
                                             AF.Square,
                                             accum_out=part[:, 1:2])
                    nc.tensor.matmul(psg[:], gm_sb[k][:], part[:],
                                     start=(k == 0), stop=(k == KCH - 1))
                stat = pool.tile([NG, 2], f32r, tag=f"stat{tag}",
                                 name=f"stat{tag}")
                with nc.allow_low_precision(reason="4B"):
                    nc.vector.tensor_copy(stat[:], psg[:])
                return stat

            def gn_finalize(pss, pool, tag):
                rm = pool.tile([NG, 2], f32r, tag=f"rm{tag}", name=f"rm{tag}")
                ex2 = pool.tile([NG, 1], f32, tag=f"ex2{tag}", name=f"ex2{tag}")
                vneg = pool.tile([NG, 1], f32, tag=f"vn{tag}", name=f"vn{tag}")
                std = pool.tile([NG, 1], f32, tag=f"std{tag}", name=f"std{tag}")
                with nc.allow_low_precision(reason="4B"):
                    nc.vector.tensor_scalar_mul(rm[:, 1:2], pss[:, 0:1],
                                                1.0 / NSTAT)
                    nc.vector.tensor_scalar_mul(ex2[:], pss[:, 1:2],
                                                1.0 / NSTAT)
                    nc.vector.scalar_tensor_tensor(
                        vneg[:], rm[:, 1:2].bitcast(f32), rm[:, 1:2].bitcast(f32),
                        ex2[:], op0=AluOpType.mult, op1=AluOpType.subtract)
                    nc.scalar.activation(std[:], vneg[:], AF.Sqrt,
                                         bias=eps_sb[:], scale=-1.0)
                    nc.vector.reciprocal(rm[:, 0:1], std[:])
                return rm

            def gn_combine(ag_chunks, msel, pool, tag):
                """sum the relevant rank partials -> rm [NG,2]=(rstd, mean)."""
                pss = ps.tile([NG, 2], f32, tag="psB", bufs=4)
                for c2 in range(2):
                    nc.tensor.matmul(pss[:], msel[c2][:], ag_chunks[c2][:],
                                     start=(c2 == 0), stop=(c2 == 1))
                return gn_finalize(pss, pool, tag)

            def gn_expand(rm, lnw, lnb, pool, tag, want_t_f32r):
                """per-channel scale s [128,1]x8 (f32) and shift t."""
                s_l, t_l = [], []
                for k in range(KCH):
                    pse = ps.tile([128, 2], f32, tag="psB", bufs=4)
                    nc.tensor.matmul(pse[:], gmT_sb[k][:], rm[:],
                                     start=True, stop=True)
                    s_k = pool.tile([128, 1], f32, tag=f"s{tag}{k}",
                                    name=f"s{tag}{k}")
                    nc.vector.tensor_tensor(s_k[:], pse[:, 0:1], lnw[k][:],
                                            AluOpType.mult)
                    if want_t_f32r:
                        sneg = pool.tile([128, 1], f32, tag=f"sn{tag}{k}",
                                         name=f"sn{tag}{k}")
                        nc.vector.tensor_scalar_mul(sneg[:], s_k[:], -1.0)
                        t_k = pool.tile([128, 2], f32r, tag=f"t{tag}{k}",
                                        name=f"t{tag}{k}")
                        with nc.allow_low_precision(reason="4B"):
                            nc.scalar.activation(t_k[:, 0:1], pse[:, 1:2],
                                                 AF.Identity,
                                                 bias=lnb[k][:], scale=sneg[:])
                            nc.scalar.activation(t_k[:, 1:2], pse[:, 1:2],
                                                 AF.Identity,
                                                 bias=0.0, scale=0.0)
                    else:
                        tmp = pool.tile([128, 1], f32, tag=f"tm{tag}{k}",
                                        name=f"tm{tag}{k}")
                        nc.vector.tensor_tensor(tmp[:], pse[:, 1:2], s_k[:],
                                                AluOpType.mult)
                        t_k = pool.tile([128, 1], f32, tag=f"t{tag}{k}",
                                        name=f"t{tag}{k}")
                        nc.vector.tensor_tensor(t_k[:], lnb[k][:], tmp[:],
                                                AluOpType.subtract)
                    s_l.append(s_k)
                    t_l.append(t_k)
                return s_l, t_l

            a2a_in = dram.tile([NC, 128, SLICE], f32r, tag="a2ai")
            a2a_out = dram.tile([NC, 128, SLICE], f32r, tag="a2ao")
            ag1_in = dram.tile([NG, 2], f32r, tag="ag1i")
            ag1_out = dram.tile([NC * NG, 2], f32r, tag="ag1o")
            ag2_in = dram.tile([NG, 2], f32r, tag="ag2i")
            ag2_out = dram.tile([NC * NG, 2], f32r, tag="ag2o")

            # ================= phase 1: GN1 + QKV + attention =============
            with tc.tile_pool(name="p1", bufs=1) as p1:
                stat1 = gn_partials(x_res, p1, "g1")
                nc.sync.dma_start(ag1_in[:], stat1[:])
                nc.gpsimd.collective_compute(
                    "AllGather", AluOpType.bypass, replica_groups=RG,
                    ins=[ag1_in[:].opt()], outs=[ag1_out[:].opt()])
                ag1_sb = []
                for c2 in range(2):
                    t = p1.tile([128, 2], f32r, tag=f"ag1{c2}",
                                name=f"ag1sb{c2}")
                    nc.sync.dma_start(t[:],
                                      ag1_out[128 * c2:128 * (c2 + 1), :])
                    ag1_sb.append(t)
                wqk_o, wkv_o = [], []
                for k in range(KCH):
                    t = p1.tile([128, 256], f32r, tag=f"wqko{k}",
                                name=f"wqko{k}")
                    nc.sync.dma_start(t[:], d_wqk[128 * k:128 * (k + 1), :])
                    wqk_o.append(t)
                    t2 = p1.tile([128, 256], f32r, tag=f"wkvo{k}",
                                 name=f"wkvo{k}")
                    nc.sync.dma_start(t2[:], d_wkv[128 * k:128 * (k + 1), :])
                    wkv_o.append(t2)

                wqk_b, wkv_b, bq_b = {}, {}, {}
                for bi in range(B):
                    rm = gn_combine(ag1_sb, mb_sb[f"b{bi}"], p1, f"g1b{bi}")
                    s_l, t_l = gn_expand(rm, ln1w_sb, ln1b_sb, p1,
                                         f"g1b{bi}", want_t_f32r=True)
                    wqk_b[bi], wkv_b[bi] = [], []
                    for k in range(KCH):
                        wq = p1.tile([128, 256], f32r, tag=f"wqs{k}", bufs=2,
                                     name=f"wqs{bi}{k}")
                        wv = p1.tile([128, 256], f32r, tag=f"wvs{k}", bufs=2,
                                     name=f"wvs{bi}{k}")
                        with nc.allow_low_precision(reason="4B"):
                            nc.vector.tensor_scalar_mul(wq[:], wqk_o[k][:],
                                                        s_l[k][:])
                            nc.vector.tensor_scalar_mul(wv[:], wkv_o[k][:],
                                                        s_l[k][:])
                        wqk_b[bi].append(wq)
                        wkv_b[bi].append(wv)
                    bq_b[bi] = {}
                    for nm, wo, brow in (("qk", wqk_o, bqk_sb),
                                         ("kv", wkv_o, bkv_sb)):
                        psr = ps.tile([2, 256], f32, tag="psB", bufs=4)
                        nc.tensor.matmul(psr[:], id2_sb[:], brow[:],
                                         start=True, stop=False)
                        for k in range(KCH):
                            nc.tensor.matmul(psr[:], t_l[k][:], wo[k][:],
                                             start=False,
                                             stop=(k == KCH - 1))
                        bkr = p1.tile([1, 256], f32r, tag=f"b{nm}r{bi}",
                                      name=f"b{nm}r{bi}")
                        with nc.allow_low_precision(reason="4B"):
                            nc.scalar.copy(bkr[:], psr[0:1, :])
                        bq_b[bi][nm] = bkr

                # ---------- QKV sweeps ----------
                qk_sb, kv_sb = {}, {}
                for bi in range(B):
                    qk_sb[bi] = [
                        p1.tile([128, T_b], f32r, tag=f"qk{bi}{m}",
                                name=f"qk{bi}{m}")
                        for m in range(2)]
                    kvt = p1.tile([128, NKT * 258], f32r, tag=f"kv{bi}",
                                  name=f"kv{bi}")
                    kv_sb[bi] = kvt
                    for th in range(2):
                        ps_qk = [ps.tile([128, QT], f32, tag="psA", bufs=4,
                                         name=f"psqk{bi}{th}{i}")
                                 for i in range(4)]
                        ps_kv = [ps.tile([128, 512], f32, tag="psB", bufs=4,
                                         name=f"pskv{bi}{th}{i}")
                                 for i in range(NKT // 4)]
                        for k in range(KCH):
                            xts = p1.tile([128, THL], f32r, tag="xts", bufs=3,
                                          name=f"xts{bi}{th}{k}")
                            nc.sync.dma_start(
                                xts[:], d_xT[128 * k:128 * (k + 1),
                                             bi * T_b + th * THL:
                                             bi * T_b + (th + 1) * THL])
                            for m in range(2):
                                for tq in range(2):
                                    nc.tensor.matmul(
                                        ps_qk[2 * m + tq][:],
                                        wqk_b[bi][k][:, 128 * m:128 * (m + 1)],
                                        xts[:, QT * tq:QT * (tq + 1)],
                                        start=(k == 0), stop=False)
                            for j in range(NKT // 4):
                                for h2 in range(2):
                                    nc.tensor.matmul(
                                        ps_kv[j][:, 256 * h2:256 * (h2 + 1)],
                                        xts[:, 128 * (2 * j + h2):
                                            128 * (2 * j + h2 + 1)],
                                        wkv_b[bi][k][:],
                                        start=(k == 0 and h2 == 0),
                                        stop=False)
                        for m in range(2):
                            for tq in range(2):
                                nc.tensor.matmul(
                                    ps_qk[2 * m + tq][:],
                                    bq_b[bi]["qk"][0:1, 128 * m:128 * (m + 1)],
                                    ones_c[0:1, 0:QT],
                                    start=False, stop=True)
                        for j in range(NKT // 4):
                            for h2 in range(2):
                                nc.tensor.matmul(
                                    ps_kv[j][:, 256 * h2:256 * (h2 + 1)],
                                    ones_c[0:1, 0:128], bq_b[bi]["kv"][:],
                                    start=False, stop=(h2 == 1))
                        for m in range(2):
                            for tq in range(2):
                                with nc.allow_low_precision(reason="4B"):
                                    nc.scalar.copy(
                                        qk_sb[bi][m][:,
                                                     th * THL + QT * tq:
                                                     th * THL + QT * (tq + 1)],
                                        ps_qk[2 * m + tq][:])
                        for j in range(NKT // 4):
                            for h2 in range(2):
                                tcg = th * (NKT // 2) + 2 * j + h2
                                base = 258 * tcg
                                src = ps_kv[j][:, 256 * h2:256 * h2 + 256]
                                with nc.allow_low_precision(reason="4B"):
                                    nc.vector.tensor_copy(
                                        kvt[:, base:base + 128], src[:, 0:128])
                                    nc.vector.tensor_copy(
                                        kvt[:, base + 128:base + 192],
                                        src[:, 128:192])
                                    nc.vector.tensor_copy(
                                        kvt[:, base + 193:base + 257],
                                        src[:, 192:256])
                    onesf = p1.tile([128, NKT], f32, tag="onesf", bufs=2,
                                    name=f"onesf{bi}")
                    nc.vector.memset(onesf[:], 1.0)
                    kv3 = kvt[:].rearrange("p (a c) -> p a c", c=258)
                    with nc.allow_low_precision(reason="4B"):
                        nc.scalar.activation(kv3[:, :, 192:193],
                                             onesf[:].unsqueeze(-1), AF.Copy)
                        nc.scalar.activation(kv3[:, :, 257:258],
                                             onesf[:].unsqueeze(-1), AF.Copy)
                    for hl in range(HPC):
                        dstk = d_pres[0, bi, hl].rearrange(
                            "(a p) d -> p a d", p=128)
                        nc.sync.dma_start(
                            dstk, kv3[:, :, 64 * hl:64 * (hl + 1)].bitcast(f32))
                        dstv = d_pres[1, bi, hl].rearrange(
                            "(a p) d -> p a d", p=128)
                        off = 128 + 65 * hl
                        nc.sync.dma_start(
                            dstv, kv3[:, :, off:off + 64].bitcast(f32))

                # ---------- attention ----------
                for bi in range(B):
                    for hl in range(HPC):
                        lo, hi = 64 * hl, 64 * (hl + 1)
                        for qj in range(4):
                            ps_a = ps.tile([65, QT], f32, tag="psB", bufs=4,
                                           name=f"psa{bi}{hl}{qj}")
                            nkt_q = DI * (qj + 1)
                            for ki in range(nkt_q):
                                ps_l = ps.tile([128, QT], f32, tag="psA",
                                               bufs=4,
                                               name=f"psl{bi}{hl}{qj}{ki}")
                                nc.tensor.matmul(
                                    ps_l[:],
                                    qk_sb[bi][1][lo:hi,
                                                 128 * ki:128 * (ki + 1)],
                                    qk_sb[bi][0][lo:hi,
                                                 QT * qj:QT * (qj + 1)],
                                    start=True, stop=True)
                                di = ki - DI * qj
                                if di >= 0:
                                    nc.vector.tensor_tensor(
                                        ps_l[:], ps_l[:], cm_sb[di][:],
                                        AluOpType.add)
                                pt = p1.tile([128, QT], f32r, tag="pt",
                                             bufs=3,
                                             name=f"pt{bi}{hl}{qj}{ki}")
                                with nc.allow_low_precision(reason="4B"):
                                    nc.scalar.activation(pt[:], ps_l[:],
                                                         AF.Exp)
                                base = 258 * ki + 128 + 65 * hl
                                nc.tensor.matmul(
                                    ps_a[:], kv_sb[bi][:, base:base + 65],
                                    pt[:],
                                    start=(ki == 0), stop=(ki == nkt_q - 1))
                            rec = p1.tile([65, QT], f32r, tag="rec", bufs=2,
                                          name=f"rec{bi}{hl}{qj}")
                            with nc.allow_low_precision(reason="4B"):
                                nc.vector.reciprocal(rec[64:65, :],
                                                     ps_a[64:65, :])
                            ps_bc = ps.tile([64, QT], f32, tag="psB", bufs=4,
                                            name=f"psbc{bi}{hl}{qj}")
                            nc.tensor.matmul(ps_bc[:], ones_c[64:65, 0:64],
                                             rec[64:65, :],
                                             start=True, stop=True)
                            bc = p1.tile([64, QT], f32, tag="bc", bufs=2,
                                         name=f"bc{bi}{hl}{qj}")
                            nc.scalar.copy(bc[:], ps_bc[:])
                            astg = p1.tile([64, QT], f32r, tag="astg", bufs=3,
                                           name=f"astg{bi}{hl}{qj}")
                            with nc.allow_low_precision(reason="4B"):
                                nc.vector.tensor_tensor(astg[:],
                                                        ps_a[0:64, :],
                                                        bc[:], AluOpType.mult)
                            nc.sync.dma_start(
                                a2a_in[4 * bi + qj, lo:hi, :], astg[:])

            nc.gpsimd.collective_compute(
                "AllToAll", AluOpType.bypass, replica_groups=RG,
                ins=[a2a_in[:].opt()], outs=[a2a_out[:].opt()])

            # ================= phase 2: proj + GN2 + MLP ==================
            with tc.tile_pool(name="p2", bufs=1) as p2:
                a_sb = []
                for j in range(NC):
                    t = p2.tile([128, SLICE], f32r, tag="a", bufs=8,
                                name=f"a{j}")
                    nc.sync.dma_start(t[:], a2a_out[j])
                    a_sb.append(t)

                x1_sb = []
                for mb in range(2):
                    ps_x = [ps.tile([128, SLICE], f32, tag="psA", bufs=4,
                                    name=f"psx{mb}{i}") for i in range(4)]
                    for k in range(KCH):
                        wt = p2.tile([128, 512], f32r, tag="ws", bufs=5,
                                     name=f"wsp{mb}{k}")
                        nc.sync.dma_start(
                            wt[:], d_wap[128 * k:128 * (k + 1),
                                         512 * mb:512 * (mb + 1)])
                        for m in range(4):
                            nc.tensor.matmul(
                                ps_x[m][:], wt[:, 128 * m:128 * (m + 1)],
                                a_sb[k][:],
                                start=(k == 0), stop=(k == KCH - 1))
                    for m in range(4):
                        gm_i = 4 * mb + m
                        x1 = x1p.tile([128, SLICE], f32, tag=f"x1_{gm_i}",
                                      name=f"x1_{gm_i}")
                        nc.vector.scalar_tensor_tensor(
                            x1[:], ps_x[m][:], bap_sb[gm_i][:],
                            x_res[gm_i][:],
                            op0=AluOpType.add, op1=AluOpType.add)
                        x1_sb.append(x1)

                # GN2
                stat2 = gn_partials(x1_sb, p2, "g2")
                nc.sync.dma_start(ag2_in[:], stat2[:])
                nc.gpsimd.collective_compute(
                    "AllGather", AluOpType.bypass, replica_groups=RG,
                    ins=[ag2_in[:].opt()], outs=[ag2_out[:].opt()])
                ag2_sb = []
                for c2 in range(2):
                    t = p2.tile([128, 2], f32r, tag=f"ag2{c2}",
                                name=f"ag2sb{c2}")
                    nc.sync.dma_start(t[:],
                                      ag2_out[128 * c2:128 * (c2 + 1), :])
                    ag2_sb.append(t)
                rm2 = gn_combine(ag2_sb, mb_sb["my"], p2, "g2")
                s2_l, t2_l = gn_expand(rm2, ln2w_sb, ln2b_sb, p2, "g2",
                                       want_t_f32r=False)
                x1n_sb = []
                for k in range(KCH):
                    xn = p2.tile([128, SLICE], f32r, tag="x1n", bufs=8,
                                 name=f"x1n{k}")
                    with nc.allow_low_precision(reason="4B"):
                        nc.scalar.activation(xn[:], x1_sb[k][:], AF.Identity,
                                             bias=t2_l[k][:], scale=s2_l[k][:])
                    x1n_sb.append(xn)

                # fc + gelu
                h_sb = [p2.tile([128, 4 * SLICE], f32r, tag=f"h{g}",
                                name=f"h{g}")
                        for g in range(KCH)]
                for mb in range(8):
                    ps_h = [ps.tile([128, SLICE], f32, tag="psA", bufs=4,
                                    name=f"psh{mb}{i}") for i in range(4)]
                    for k in range(KCH):
                        wt = p2.tile([128, 512], f32r, tag="ws", bufs=5,
                                     name=f"wsf{mb}{k}")
                        nc.sync.dma_start(
                            wt[:], d_wfc[128 * k:128 * (k + 1),
                                         512 * mb:512 * (mb + 1)])
                        for m in range(4):
                            nc.tensor.matmul(
                                ps_h[m][:], wt[:, 128 * m:128 * (m + 1)],
                                x1n_sb[k][:],
                                start=(k == 0), stop=(k == KCH - 1))
                    for m in range(4):
                        g4 = 4 * mb + m
                        with nc.allow_low_precision(reason="4B"):
                            nc.scalar.activation(
                                h_sb[g4 // 4][:, SLICE * (g4 % 4):
                                              SLICE * (g4 % 4 + 1)],
                                ps_h[m][:], gelu_fn,
                                bias=bfc_sb[g4][:], scale=1.0)

                # mproj + residual + out
                for mb in range(2):
                    ps_m = [ps.tile([128, SLICE], f32, tag="psA", bufs=4,
                                    name=f"psm{mb}{i}") for i in range(4)]
                    for k in range(H4 // 128):
                        wt = p2.tile([128, 512], f32r, tag="ws", bufs=5,
                                     name=f"wsm{mb}{k}")
                        nc.sync.dma_start(
                            wt[:], d_wmp[128 * k:128 * (k + 1),
                                         512 * mb:512 * (mb + 1)])
                        rhs = h_sb[k // 4][:, SLICE * (k % 4):
                                           SLICE * (k % 4 + 1)]
                        for m in range(4):
                            nc.tensor.matmul(
                                ps_m[m][:], wt[:, 128 * m:128 * (m + 1)],
                                rhs,
                                start=(k == 0), stop=(k == H4 // 128 - 1))
                    for m in range(4):
                        gm_i = 4 * mb + m
                        x2 = p2.tile([128, SLICE], f32, tag="x2", bufs=3,
                                     name=f"x2_{gm_i}")
                        nc.vector.scalar_tensor_tensor(
                            x2[:], ps_m[m][:], bmp_sb[gm_i][:],
                            x1_sb[gm_i][:],
                            op0=AluOpType.add, op1=AluOpType.add)
                        nc.sync.dma_start(
                            d_out[128 * gm_i:128 * (gm_i + 1), :], x2[:])

    nc.compile()
    return nc


# ------------------------------------------------------------------
# host side
# ------------------------------------------------------------------

_CACHE = {}


def _get_nc(T_b):
    if T_b not in _CACHE:
        _CACHE[T_b] = build(T_b)
    return _CACHE[T_b]


def make_in_maps(x, w_attn, b_attn, w_aproj, b_aproj, ln1_w, ln1_b,
                 ln2_w, ln2_b, w_fc, b_fc, w_mproj, b_mproj):
    Bx, T_b, Hx = x.shape
    QT = T_b // 4
    SLICE = Bx * T_b // NC
    DI = QT // 128
    f = np.float32

    xT = np.ascontiguousarray(
        x.transpose(2, 0, 1).reshape(Hx, Bx * T_b), dtype=f)

    gmask = np.zeros((KCH, 128, NG), f)
    for k in range(KCH):
        for p in range(128):
            gmask[k, p, (128 * k + p) // (Hx // NG)] = 1.0
    gmaskT = np.ascontiguousarray(gmask.transpose(0, 2, 1))

    def msel(ranks):
        m = np.zeros((2, 128, NG), f)
        for r in ranks:
            for g in range(NG):
                row = NG * r + g
                m[row // 128, row % 128, g] = 1.0
        return m

    mb0 = msel(range(4))
    mb1 = msel(range(4, 8))

    cmask = np.zeros((DI, 128, QT), f)
    for di in range(DI):
        kk = np.arange(128)[:, None] + 128 * di
        qq = np.arange(QT)[None, :]
        cmask[di] = np.where(kk <= qq, 0.0, -1e10)

    ones_c = np.ones((128, QT), f)
    id2 = np.eye(2, dtype=f)
    hsel = np.zeros((KCH, 16, 128), f)
    for k in range(KCH):
        for p in range(128):
            hsel[k, (128 * k + p) // 64, p] = 1.0

    in_maps = []
    for c in range(NC):
        h0 = HPC * c
        qcols = np.concatenate(
            [np.arange(64 * (h0 + i), 64 * (h0 + i + 1)) for i in range(HPC)])
        kcols = Hx + qcols
        vcols = 2 * Hx + qcols
        wqk = np.ascontiguousarray(
            np.concatenate([w_attn[:, qcols], w_attn[:, kcols]], axis=1),
            dtype=f)
        wkv = np.ascontiguousarray(
            np.concatenate([w_attn[:, kcols], w_attn[:, vcols]], axis=1),
            dtype=f)
        bqk = np.zeros((2, 256), f)
        bqk[0] = np.concatenate([b_attn[qcols], b_attn[kcols]])
        bkv = np.zeros((2, 256), f)
        bkv[0] = np.concatenate([b_attn[kcols], b_attn[vcols]])
        in_maps.append({
            "xT": xT,
            "x_res": np.ascontiguousarray(xT[:, SLICE * c:SLICE * (c + 1)]),
            "wqk": wqk, "wkv": wkv,
            "bqk": np.ascontiguousarray(bqk, f),
            "bkv": np.ascontiguousarray(bkv, f),
            "w_aproj": np.ascontiguousarray(w_aproj, f),
            "b_aproj": np.asarray(b_aproj, f).reshape(Hx, 1),
            "w_fc": np.ascontiguousarray(w_fc, f),
            "b_fc": np.asarray(b_fc, f).reshape(4 * Hx, 1),
            "w_mproj": np.ascontiguousarray(w_mproj, f),
            "b_mproj": np.asarray(b_mproj, f).reshape(Hx, 1),
            "ln1w": np.asarray(ln1_w, f).reshape(Hx, 1),
            "ln1b": np.asarray(ln1_b, f).reshape(Hx, 1),
            "ln2w": np.asarray(ln2_w, f).reshape(Hx, 1),
            "ln2b": np.asarray(ln2_b, f).reshape(Hx, 1),
            "gmask": gmask, "gmaskT": gmaskT,
            "Mb0": mb0, "Mb1": mb1,
            "Mmy": mb0 if c < 4 else mb1,
            "hsel": hsel,
            "cmaskA": cmask,
            "ones_c": ones_c,
            "id2": id2,
        })
    return in_maps


def kernel(x, w_attn, b_attn, w_aproj, b_aproj, ln1_w, ln1_b, ln2_w, ln2_b,
           w_fc, b_fc, w_mproj, b_mproj, _trace=False):
    from concourse import bass_utils

    x = np.asarray(x, np.float32)
    Bx, T_b, Hx = x.shape
    SLICE = Bx * T_b // NC

    nc = _get_nc(T_b)
    in_maps = make_in_maps(
        x, np.asarray(w_attn, np.float32), np.asarray(b_attn, np.float32),
        np.asarray(w_aproj, np.float32), np.asarray(b_aproj, np.float32),
        np.asarray(ln1_w, np.float32), np.asarray(ln1_b, np.float32),
        np.asarray(ln2_w, np.float32), np.asarray(ln2_b, np.float32),
        np.asarray(w_fc, np.float32), np.asarray(b_fc, np.float32),
        np.asarray(w_mproj, np.float32), np.asarray(b_mproj, np.float32))

    res = bass_utils.run_bass_kernel_spmd(
        nc, in_maps, core_ids=list(range(NC)), trace=_trace)

    xT_out = np.empty((Hx, Bx * T_b), np.float32)
    present = np.empty((2, Bx, NH, T_b, HD), np.float32)
    for c in range(NC):
        r = res.results[c]
        xT_out[:, SLICE * c:SLICE * (c + 1)] = r["x2T"]
        present[:, :, HPC * c:HPC * (c + 1)] = r["present"]
    x_out = np.ascontiguousarray(
        xT_out.reshape(Hx, Bx, T_b).transpose(1, 2, 0))
    kernel.last_exec_time_ns = res.exec_time_ns
    kernel.last_results = res
    return (x_out, present)


# revision 18
# speedup vs baseline: 1.1150x; 1.1150x over previous
"""Fused transformer block (GN -> causal MHA -> GN -> MLP, residuals) on
8 Trainium2 NeuronCores.

Sharding: 8-way tensor parallel over the 16 attention heads (2 heads per
core, both batch elements on every core); everything after attention is
sequence(T)-sharded (each core owns a 512-column slice of the flattened
[B*T] axis). Cross-core traffic: one tiny stats AllGather per GroupNorm
(partial sums of per-(batch,group) statistics) and one 2MB AllToAll that
re-shards attention outputs from head-sharded to T-sharded. Weights for
the T-sharded matmuls (w_aproj/w_fc/w_mproj) are streamed from HBM.

All matmuls run as fp32r (full-rate fp32 with PE-internal mantissa
rounding); GroupNorm scale/shift is folded into the QKV weights so the
16MB activation tensor is read straight into the tensor engine.
"""

import os
import sys

sys.path.insert(0, "/opt/trn_rl_repo")

import numpy as np

import concourse.bass as bass
import concourse.bacc as bacc
import concourse.mybir as mybir
import concourse.tile as tile
from concourse.alu_op_type import AluOpType

f32 = mybir.dt.float32
f32r = mybir.dt.float32r
AF = mybir.ActivationFunctionType
AX = mybir.AxisListType

NC = 8          # cores
B = 2
H = 1024
NH = 16
HD = 64
NG = 32         # groupnorm groups
H4 = 4 * H
EPS = 1e-05
HPC = NH // NC  # heads per core = 2
KCH = H // 128  # 8 input-channel chunks


def build(T_b, sim_safe_gelu=False):
    """Build + bacc-compile the SPMD kernel for per-batch seq len T_b."""
    gelu_fn = AF.Sigmoid if sim_safe_gelu else AF.Gelu_apprx_tanh
    QT = T_b // 4          # q-tile width == per-core T slice
    SLICE = B * T_b // NC  # columns of the flattened B*T axis per core
    assert SLICE == QT
    NKT = T_b // 128       # k-tiles per batch
    THL = T_b // 2         # half-batch (per qkv sweep)
    DI = QT // 128         # diagonal straddle patterns
    NSTAT = T_b * (H // NG)
    RG = [list(range(NC))]

    nc = bacc.Bacc("TRN2", target_bir_lowering=False, debug=False,
                   num_devices=NC)

    # ---- DRAM I/O ----
    d_xT = nc.dram_tensor("xT", [H, B * T_b], f32r, kind="ExternalInput")
    d_xres = nc.dram_tensor("x_res", [H, SLICE], f32, kind="ExternalInput")
    d_wqk = nc.dram_tensor("wqk", [H, 256], f32r, kind="ExternalInput")
    d_wkv = nc.dram_tensor("wkv", [H, 256], f32r, kind="ExternalInput")
    d_bqk = nc.dram_tensor("bqk", [2, 256], f32r, kind="ExternalInput")
    d_bkv = nc.dram_tensor("bkv", [2, 256], f32r, kind="ExternalInput")
    d_id2 = nc.dram_tensor("id2", [2, 2], f32r, kind="ExternalInput")
    d_wap = nc.dram_tensor("w_aproj", [H, H], f32r, kind="ExternalInput")
    d_bap = nc.dram_tensor("b_aproj", [H, 1], f32, kind="ExternalInput")
    d_wfc = nc.dram_tensor("w_fc", [H, H4], f32r, kind="ExternalInput")
    d_bfc = nc.dram_tensor("b_fc", [H4, 1], f32, kind="ExternalInput")
    d_wmp = nc.dram_tensor("w_mproj", [H4, H], f32r, kind="ExternalInput")
    d_bmp = nc.dram_tensor("b_mproj", [H, 1], f32, kind="ExternalInput")
    d_ln1w = nc.dram_tensor("ln1w", [H, 1], f32, kind="ExternalInput")
    d_ln1b = nc.dram_tensor("ln1b", [H, 1], f32, kind="ExternalInput")
    d_ln2w = nc.dram_tensor("ln2w", [H, 1], f32, kind="ExternalInput")
    d_ln2b = nc.dram_tensor("ln2b", [H, 1], f32, kind="ExternalInput")
    d_gm = nc.dram_tensor("gmask", [KCH, 128, NG], f32r, kind="ExternalInput")
    d_gmT = nc.dram_tensor("gmaskT", [KCH, NG, 128], f32r, kind="ExternalInput")
    d_mb0 = nc.dram_tensor("Mb0", [2, 128, NG], f32r, kind="ExternalInput")
    d_mb1 = nc.dram_tensor("Mb1", [2, 128, NG], f32r, kind="ExternalInput")
    d_mmy = nc.dram_tensor("Mmy", [2, 128, NG], f32r, kind="ExternalInput")
    d_hsel = nc.dram_tensor("hsel", [KCH, 16, 128], f32r, kind="ExternalInput")
    d_cmask = nc.dram_tensor("cmaskA", [DI, 128, QT], f32, kind="ExternalInput")
    d_ones = nc.dram_tensor("ones_c", [128, QT], f32r, kind="ExternalInput")

    d_out = nc.dram_tensor("x2T", [H, SLICE], f32, kind="ExternalOutput")
    d_pres = nc.dram_tensor("present", [2, B, HPC, T_b, HD], f32,
                            kind="ExternalOutput")

    with tile.TileContext(nc) as tc:
        with (
            tc.tile_pool(name="ctp", bufs=1) as ctp,     # constants
            tc.tile_pool(name="vecp", bufs=1) as vecp,   # [128,1] vectors
            tc.tile_pool(name="xrp", bufs=1) as xrp,     # x residual slice
            tc.tile_pool(name="x1p", bufs=1) as x1p,     # x1 slice
            tc.tile_pool(name="smp", bufs=1) as smp,     # small stat tiles
            tc.tile_pool(name="ps", bufs=4, space="PSUM") as ps,
            tc.tile_pool(name="dram", bufs=1, space="DRAM") as dram,
        ):
            # ---------- constants ----------
            ones_c = ctp.tile([128, QT], f32r, tag="ones_c")
            nc.sync.dma_start(ones_c[:], d_ones[:])
            cm_sb = []
            for di in range(DI):
                t = ctp.tile([128, QT], f32, tag=f"cm{di}", name=f"cm{di}")
                nc.sync.dma_start(t[:], d_cmask[di])
                cm_sb.append(t)
            gm_sb, gmT_sb = [], []
            for k in range(KCH):
                t = ctp.tile([128, NG], f32r, tag=f"gm{k}", name=f"gm{k}")
                nc.sync.dma_start(t[:], d_gm[k])
                gm_sb.append(t)
                t2 = ctp.tile([NG, 128], f32r, tag=f"gmT{k}", name=f"gmT{k}")
                nc.sync.dma_start(t2[:], d_gmT[k])
                gmT_sb.append(t2)
            mb_sb = {}
            for nm, dt_ in (("b0", d_mb0), ("b1", d_mb1), ("my", d_mmy)):
                mb_sb[nm] = []
                for c2 in range(2):
                    t = ctp.tile([128, NG], f32r, tag=f"M{nm}{c2}",
                                 name=f"M{nm}{c2}")
                    nc.sync.dma_start(t[:], dt_[c2])
                    mb_sb[nm].append(t)

            def vec_chunks(dten, n, tag):
                out = []
                for k in range(n):
                    t = vecp.tile([128, 1], dten.dtype, tag=f"{tag}{k}",
                                  name=f"{tag}{k}")
                    nc.sync.dma_start(t[:], dten[128 * k:128 * (k + 1), :])
                    out.append(t)
                return out

            ln1w_sb = vec_chunks(d_ln1w, KCH, "ln1w")
            ln1b_sb = vec_chunks(d_ln1b, KCH, "ln1b")
            ln2w_sb = vec_chunks(d_ln2w, KCH, "ln2w")
            ln2b_sb = vec_chunks(d_ln2b, KCH, "ln2b")
            bap_sb = vec_chunks(d_bap, KCH, "bap")
            bfc_sb = vec_chunks(d_bfc, H4 // 128, "bfc")
            bmp_sb = vec_chunks(d_bmp, KCH, "bmp")
            bqk_sb = ctp.tile([2, 256], f32r, tag="bqk")
            nc.sync.dma_start(bqk_sb[:], d_bqk[:])
            bkv_sb = ctp.tile([2, 256], f32r, tag="bkv")
            nc.sync.dma_start(bkv_sb[:], d_bkv[:])
            id2_sb = ctp.tile([2, 2], f32r, tag="id2")
            nc.sync.dma_start(id2_sb[:], d_id2[:])
            eps_sb = ctp.tile([NG, 1], f32, tag="eps")
            nc.vector.memset(eps_sb[:], EPS)

            x_res = []
            for k in range(KCH):
                t = xrp.tile([128, SLICE], f32, tag=f"xres{k}",
                             name=f"xres{k}")
                nc.sync.dma_start(t[:], d_xres[128 * k:128 * (k + 1), :])
                x_res.append(t)

            # ---------- GroupNorm helpers ----------
            def gn_partials(src_chunks, pool, tag):
                """[NG,2] (sum, sumsq) partials over this core's slice."""
                psg = ps.tile([NG, 2], f32, tag="psB", bufs=4)
                for k in range(KCH):
                    part = pool.tile([128, 2], f32r, tag=f"part{tag}", bufs=2,
                                     name=f"part{tag}{k}")
                    scr = pool.tile([128, SLICE], f32, tag=f"scr{tag}", bufs=2,
                                    name=f"scr{tag}{k}")
                    with nc.allow_low_precision(reason="f32r is 4B fp32"):
                        nc.vector.reduce_sum(part[:, 0:1], src_chunks[k][:],
                                             axis=AX.X)
                        nc.scalar.activation(scr[:], src_chunks[k][:],
                                             AF.Square,
                                             accum_out=part[:, 1:2])
                    nc.tensor.matmul(psg[:], gm_sb[k][:], part[:],
                                     start=(k == 0), stop=(k == KCH - 1))
                stat = pool.tile([NG, 2], f32r, tag=f"stat{tag}",
                                 name=f"stat{tag}")
                with nc.allow_low_precision(reason="4B"):
                    nc.vector.tensor_copy(stat[:], psg[:])
                return stat

            def gn_finalize(pss, pool, tag):
                rm = pool.tile([NG, 2], f32r, tag=f"rm{tag}", name=f"rm{tag}")
                ex2 = pool.tile([NG, 1], f32, tag=f"ex2{tag}", name=f"ex2{tag}")
                vneg = pool.tile([NG, 1], f32, tag=f"vn{tag}", name=f"vn{tag}")
                std = pool.tile([NG, 1], f32, tag=f"std{tag}", name=f"std{tag}")
                with nc.allow_low_precision(reason="4B"):
                    nc.vector.tensor_scalar_mul(rm[:, 1:2], pss[:, 0:1],
                                                1.0 / NSTAT)
                    nc.vector.tensor_scalar_mul(ex2[:], pss[:, 1:2],
                                                1.0 / NSTAT)
                    nc.vector.scalar_tensor_tensor(
                        vneg[:], rm[:, 1:2].bitcast(f32), rm[:, 1:2].bitcast(f32),
                        ex2[:], op0=AluOpType.mult, op1=AluOpType.subtract)
                    nc.scalar.activation(std[:], vneg[:], AF.Sqrt,
                                         bias=eps_sb[:], scale=-1.0)
                    nc.vector.reciprocal(rm[:, 0:1], std[:])
                return rm

            def gn_combine(ag_chunks, msel, pool, tag):
                """sum the relevant rank partials -> rm [NG,2]=(rstd, mean)."""
                pss = ps.tile([NG, 2], f32, tag="psB", bufs=4)
                for c2 in range(2):
                    nc.tensor.matmul(pss[:], msel[c2][:], ag_chunks[c2][:],
                                     start=(c2 == 0), stop=(c2 == 1))
                return gn_finalize(pss, pool, tag)

            def gn_expand(rm, lnw, lnb, pool, tag, want_t_f32r):
                """per-channel scale s [128,1]x8 (f32) and shift t."""
                s_l, t_l = [], []
                for k in range(KCH):
                    pse = ps.tile([128, 2], f32, tag="psB", bufs=4)
                    nc.tensor.matmul(pse[:], gmT_sb[k][:], rm[:],
                                     start=True, stop=True)
                    s_k = pool.tile([128, 1], f32, tag=f"s{tag}{k}",
                                    name=f"s{tag}{k}")
                    nc.vector.tensor_tensor(s_k[:], pse[:, 0:1], lnw[k][:],
                                            AluOpType.mult)
                    if want_t_f32r:
                        sneg = pool.tile([128, 1], f32, tag=f"sn{tag}{k}",
                                         name=f"sn{tag}{k}")
                        nc.vector.tensor_scalar_mul(sneg[:], s_k[:], -1.0)
                        t_k = pool.tile([128, 2], f32r, tag=f"t{tag}{k}",
                                        name=f"t{tag}{k}")
                        with nc.allow_low_precision(reason="4B"):
                            nc.scalar.activation(t_k[:, 0:1], pse[:, 1:2],
                                                 AF.Identity,
                                                 bias=lnb[k][:], scale=sneg[:])
                            nc.scalar.activation(t_k[:, 1:2], pse[:, 1:2],
                                                 AF.Identity,
                                                 bias=0.0, scale=0.0)
                    else:
                        tmp = pool.tile([128, 1], f32, tag=f"tm{tag}{k}",
                                        name=f"tm{tag}{k}")
                        nc.vector.tensor_tensor(tmp[:], pse[:, 1:2], s_k[:],
                                                AluOpType.mult)
                        t_k = pool.tile([128, 1], f32, tag=f"t{tag}{k}",
                                        name=f"t{tag}{k}")
                        nc.vector.tensor_tensor(t_k[:], lnb[k][:], tmp[:],
                                                AluOpType.subtract)
                    s_l.append(s_k)
                    t_l.append(t_k)
                return s_l, t_l

            a2a_in = dram.tile([NC, 128, SLICE], f32r, tag="a2ai")
            a2a_out = dram.tile([NC, 128, SLICE], f32r, tag="a2ao")
            ag1_in = dram.tile([NG, 2], f32r, tag="ag1i")
            ag1_out = dram.tile([NC * NG, 2], f32r, tag="ag1o")
            ag2_in = dram.tile([NG, 2], f32r, tag="ag2i")
            ag2_out = dram.tile([NC * NG, 2], f32r, tag="ag2o")

            # ================= phase 1: GN1 + QKV + attention =============
            with tc.tile_pool(name="p1", bufs=1) as p1:
                stat1 = gn_partials(x_res, p1, "g1")
                nc.sync.dma_start(ag1_in[:], stat1[:])
                nc.gpsimd.collective_compute(
                    "AllGather", AluOpType.bypass, replica_groups=RG,
                    ins=[ag1_in[:].opt()], outs=[ag1_out[:].opt()])
                ag1_sb = []
                for c2 in range(2):
                    t = p1.tile([128, 2], f32r, tag=f"ag1{c2}",
                                name=f"ag1sb{c2}")
                    nc.sync.dma_start(t[:],
                                      ag1_out[128 * c2:128 * (c2 + 1), :])
                    ag1_sb.append(t)
                wqk_o, wkv_o = [], []
                for k in range(KCH):
                    t = p1.tile([128, 256], f32r, tag=f"wqko{k}",
                                name=f"wqko{k}")
                    nc.sync.dma_start(t[:], d_wqk[128 * k:128 * (k + 1), :])
                    wqk_o.append(t)
                    t2 = p1.tile([128, 256], f32r, tag=f"wkvo{k}",
                                 name=f"wkvo{k}")
                    nc.sync.dma_start(t2[:], d_wkv[128 * k:128 * (k + 1), :])
                    wkv_o.append(t2)

                wqk_b, wkv_b, bq_b = {}, {}, {}
                for bi in range(B):
                    rm = gn_combine(ag1_sb, mb_sb[f"b{bi}"], p1, f"g1b{bi}")
                    s_l, t_l = gn_expand(rm, ln1w_sb, ln1b_sb, p1,
                                         f"g1b{bi}", want_t_f32r=True)
                    wqk_b[bi], wkv_b[bi] = [], []
                    for k in range(KCH):
                        wq = p1.tile([128, 256], f32r, tag=f"wqs{k}", bufs=2,
                                     name=f"wqs{bi}{k}")
                        wv = p1.tile([128, 256], f32r, tag=f"wvs{k}", bufs=2,
                                     name=f"wvs{bi}{k}")
                        with nc.allow_low_precision(reason="4B"):
                            nc.vector.tensor_scalar_mul(wq[:], wqk_o[k][:],
                                                        s_l[k][:])
                            nc.vector.tensor_scalar_mul(wv[:], wkv_o[k][:],
                                                        s_l[k][:])
                        wqk_b[bi].append(wq)
                        wkv_b[bi].append(wv)
                    bq_b[bi] = {}
                    for nm, wo, brow in (("qk", wqk_o, bqk_sb),
                                         ("kv", wkv_o, bkv_sb)):
                        psr = ps.tile([2, 256], f32, tag="psB", bufs=4)
                        nc.tensor.matmul(psr[:], id2_sb[:], brow[:],
                                         start=True, stop=False)
                        for k in range(KCH):
                            nc.tensor.matmul(psr[:], t_l[k][:], wo[k][:],
                                             start=False,
                                             stop=(k == KCH - 1))
                        bkr = p1.tile([1, 256], f32r, tag=f"b{nm}r{bi}",
                                      name=f"b{nm}r{bi}")
                        with nc.allow_low_precision(reason="4B"):
                            nc.scalar.copy(bkr[:], psr[0:1, :])
                        bq_b[bi][nm] = bkr

                # ---------- QKV sweeps ----------
                qk_sb, kv_sb = {}, {}
                for bi in range(B):
                    qk_sb[bi] = [
                        p1.tile([128, T_b], f32r, tag=f"qk{bi}{m}",
                                name=f"qk{bi}{m}")
                        for m in range(2)]
                    kvt = p1.tile([128, NKT * 258], f32r, tag=f"kv{bi}",
                                  name=f"kv{bi}")
                    kv_sb[bi] = kvt
                    for th in range(2):
                        ps_qk = [ps.tile([128, QT], f32, tag="psA", bufs=4,
                                         name=f"psqk{bi}{th}{i}")
                                 for i in range(4)]
                        ps_kv = [ps.tile([128, 512], f32, tag="psB", bufs=4,
                                         name=f"pskv{bi}{th}{i}")
                                 for i in range(NKT // 4)]
                        for k in range(KCH):
                            xts = p1.tile([128, THL], f32r, tag="xts", bufs=3,
                                          name=f"xts{bi}{th}{k}")
                            nc.sync.dma_start(
                                xts[:], d_xT[128 * k:128 * (k + 1),
                                             bi * T_b + th * THL:
                                             bi * T_b + (th + 1) * THL])
                            for m in range(2):
                                for tq in range(2):
                                    nc.tensor.matmul(
                                        ps_qk[2 * m + tq][:],
                                        wqk_b[bi][k][:, 128 * m:128 * (m + 1)],
                                        xts[:, QT * tq:QT * (tq + 1)],
                                        start=(k == 0), stop=False)
                            for j in range(NKT // 4):
                                for h2 in range(2):
                                    nc.tensor.matmul(
                                        ps_kv[j][:, 256 * h2:256 * (h2 + 1)],
                                        xts[:, 128 * (2 * j + h2):
                                            128 * (2 * j + h2 + 1)],
                                        wkv_b[bi][k][:],
                                        start=(k == 0 and h2 == 0),
                                        stop=False)
                        for m in range(2):
                            for tq in range(2):
                                nc.tensor.matmul(
                                    ps_qk[2 * m + tq][:],
                                    bq_b[bi]["qk"][0:1, 128 * m:128 * (m + 1)],
                                    ones_c[0:1, 0:QT],
                                    start=False, stop=True)
                        for j in range(NKT // 4):
                            for h2 in range(2):
                                nc.tensor.matmul(
                                    ps_kv[j][:, 256 * h2:256 * (h2 + 1)],
                                    ones_c[0:1, 0:128], bq_b[bi]["kv"][:],
                                    start=False, stop=(h2 == 1))
                        for m in range(2):
                            for tq in range(2):
                                with nc.allow_low_precision(reason="4B"):
                                    nc.scalar.copy(
                                        qk_sb[bi][m][:,
                                                     th * THL + QT * tq:
                                                     th * THL + QT * (tq + 1)],
                                        ps_qk[2 * m + tq][:])
                        for j in range(NKT // 4):
                            for h2 in range(2):
                                tcg = th * (NKT // 2) + 2 * j + h2
                                base = 258 * tcg
                                src = ps_kv[j][:, 256 * h2:256 * h2 + 256]
                                with nc.allow_low_precision(reason="4B"):
                                    nc.vector.tensor_copy(
                                        kvt[:, base:base + 128], src[:, 0:128])
                                    nc.vector.tensor_copy(
                                        kvt[:, base + 128:base + 192],
                                        src[:, 128:192])
                                    nc.vector.tensor_copy(
                                        kvt[:, base + 193:base + 257],
                                        src[:, 192:256])
                    onesf = p1.tile([128, NKT], f32, tag="onesf", bufs=2,
                                    name=f"onesf{bi}")
                    nc.vector.memset(onesf[:], 1.0)
                    kv3 = kvt[:].rearrange("p (a c) -> p a c", c=258)
                    with nc.allow_low_precision(reason="4B"):
                        nc.scalar.activation(kv3[:, :, 192:193],
                                             onesf[:].unsqueeze(-1), AF.Copy)
                        nc.scalar.activation(kv3[:, :, 257:258],
                                             onesf[:].unsqueeze(-1), AF.Copy)
                    for hl in range(HPC):
                        dstk = d_pres[0, bi, hl].rearrange(
                            "(a p) d -> p a d", p=128)
                        nc.sync.dma_start(
                            dstk, kv3[:, :, 64 * hl:64 * (hl + 1)].bitcast(f32))
                        dstv = d_pres[1, bi, hl].rearrange(
                            "(a p) d -> p a d", p=128)
                        off = 128 + 65 * hl
                        nc.sync.dma_start(
                            dstv, kv3[:, :, off:off + 64].bitcast(f32))

                # ---------- attention ----------
                for bi in range(B):
                    for hl in range(HPC):
                        lo, hi = 64 * hl, 64 * (hl + 1)
                        for qj in range(4):
                            ps_a = ps.tile([65, QT], f32, tag="psB", bufs=4,
                                           name=f"psa{bi}{hl}{qj}")
                            nkt_q = DI * (qj + 1)
                            for ki in range(nkt_q):
                                ps_l = ps.tile([128, QT], f32, tag="psA",
                                               bufs=4,
                                               name=f"psl{bi}{hl}{qj}{ki}")
                                nc.tensor.matmul(
                                    ps_l[:],
                                    qk_sb[bi][1][lo:hi,
                                                 128 * ki:128 * (ki + 1)],
                                    qk_sb[bi][0][lo:hi,
                                                 QT * qj:QT * (qj + 1)],
                                    start=True, stop=True)
                                di = ki - DI * qj
                                pt = p1.tile([128, QT], f32r, tag="pt",
                                             bufs=3,
                                             name=f"pt{bi}{hl}{qj}{ki}")
                                with nc.allow_low_precision(reason="4B"):
                                    nc.scalar.activation(pt[:], ps_l[:],
                                                         AF.Exp)
                                if di >= 0:
                                    with nc.allow_low_precision(reason="4B"):
                                        nc.vector.tensor_tensor(
                                            pt[:], pt[:], cm_sb[di][:],
                                            AluOpType.mult)
                                base = 258 * ki + 128 + 65 * hl
                                nc.tensor.matmul(
                                    ps_a[:], kv_sb[bi][:, base:base + 65],
                                    pt[:],
                                    start=(ki == 0), stop=(ki == nkt_q - 1))
                            rec = p1.tile([65, QT], f32r, tag="rec", bufs=2,
                                          name=f"rec{bi}{hl}{qj}")
                            with nc.allow_low_precision(reason="4B"):
                                nc.vector.reciprocal(rec[64:65, :],
                                                     ps_a[64:65, :])
                            ps_bc = ps.tile([64, QT], f32, tag="psB", bufs=4,
                                            name=f"psbc{bi}{hl}{qj}")
                            nc.tensor.matmul(ps_bc[:], ones_c[64:65, 0:64],
                                             rec[64:65, :],
                                             start=True, stop=True)
                            bc = p1.tile([64, QT], f32, tag="bc", bufs=2,
                                         name=f"bc{bi}{hl}{qj}")
                            nc.scalar.copy(bc[:], ps_bc[:])
                            astg = p1.tile([64, QT], f32r, tag="astg", bufs=3,
                                           name=f"astg{bi}{hl}{qj}")
                            with nc.allow_low_precision(reason="4B"):
                                nc.vector.tensor_tensor(astg[:],
                                                        ps_a[0:64, :],
                                                        bc[:], AluOpType.mult)
                            nc.sync.dma_start(
                                a2a_in[4 * bi + qj, lo:hi, :], astg[:])

            nc.gpsimd.collective_compute(
                "AllToAll", AluOpType.bypass, replica_groups=RG,
                ins=[a2a_in[:].opt()], outs=[a2a_out[:].opt()])

            # ================= phase 2: proj + GN2 + MLP ==================
            with tc.tile_pool(name="p2", bufs=1) as p2:
                a_sb = []
                for j in range(NC):
                    t = p2.tile([128, SLICE], f32r, tag="a", bufs=8,
                                name=f"a{j}")
                    nc.sync.dma_start(t[:], a2a_out[j])
                    a_sb.append(t)

                x1_sb = []
                for mb in range(2):
                    ps_x = [ps.tile([128, SLICE], f32, tag="psA", bufs=4,
                                    name=f"psx{mb}{i}") for i in range(4)]
                    for k in range(KCH):
                        wt = p2.tile([128, 512], f32r, tag="ws", bufs=5,
                                     name=f"wsp{mb}{k}")
                        nc.sync.dma_start(
                            wt[:], d_wap[128 * k:128 * (k + 1),
                                         512 * mb:512 * (mb + 1)])
                        for m in range(4):
                            nc.tensor.matmul(
                                ps_x[m][:], wt[:, 128 * m:128 * (m + 1)],
                                a_sb[k][:],
                                start=(k == 0), stop=(k == KCH - 1))
                    for m in range(4):
                        gm_i = 4 * mb + m
                        x1 = x1p.tile([128, SLICE], f32, tag=f"x1_{gm_i}",
                                      name=f"x1_{gm_i}")
                        nc.vector.scalar_tensor_tensor(
                            x1[:], ps_x[m][:], bap_sb[gm_i][:],
                            x_res[gm_i][:],
                            op0=AluOpType.add, op1=AluOpType.add)
                        x1_sb.append(x1)

                # GN2
                stat2 = gn_partials(x1_sb, p2, "g2")
                nc.sync.dma_start(ag2_in[:], stat2[:])
                nc.gpsimd.collective_compute(
                    "AllGather", AluOpType.bypass, replica_groups=RG,
                    ins=[ag2_in[:].opt()], outs=[ag2_out[:].opt()])
                ag2_sb = []
                for c2 in range(2):
                    t = p2.tile([128, 2], f32r, tag=f"ag2{c2}",
                                name=f"ag2sb{c2}")
                    nc.sync.dma_start(t[:],
                                      ag2_out[128 * c2:128 * (c2 + 1), :])
                    ag2_sb.append(t)
                rm2 = gn_combine(ag2_sb, mb_sb["my"], p2, "g2")
                s2_l, t2_l = gn_expand(rm2, ln2w_sb, ln2b_sb, p2, "g2",
                                       want_t_f32r=False)
                x1n_sb = []
                for k in range(KCH):
                    xn = p2.tile([128, SLICE], f32r, tag="x1n", bufs=8,
                                 name=f"x1n{k}")
                    with nc.allow_low_precision(reason="4B"):
                        nc.scalar.activation(xn[:], x1_sb[k][:], AF.Identity,
                                             bias=t2_l[k][:], scale=s2_l[k][:])
                    x1n_sb.append(xn)

                # fc + gelu
                h_sb = [p2.tile([128, 4 * SLICE], f32r, tag=f"h{g}",
                                name=f"h{g}")
                        for g in range(KCH)]
                for mb in range(8):
                    ps_h = [ps.tile([128, SLICE], f32, tag="psA", bufs=4,
                                    name=f"psh{mb}{i}") for i in range(4)]
                    for k in range(KCH):
                        wt = p2.tile([128, 512], f32r, tag="ws", bufs=5,
                                     name=f"wsf{mb}{k}")
                        nc.sync.dma_start(
                            wt[:], d_wfc[128 * k:128 * (k + 1),
                                         512 * mb:512 * (mb + 1)])
                        for m in range(4):
                            nc.tensor.matmul(
                                ps_h[m][:], wt[:, 128 * m:128 * (m + 1)],
                                x1n_sb[k][:],
                                start=(k == 0), stop=(k == KCH - 1))
                    for m in range(4):
                        g4 = 4 * mb + m
                        with nc.allow_low_precision(reason="4B"):
                            nc.scalar.activation(
                                h_sb[g4 // 4][:, SLICE * (g4 % 4):
                                              SLICE * (g4 % 4 + 1)],
                                ps_h[m][:], gelu_fn,
                                bias=bfc_sb[g4][:], scale=1.0)

                # mproj + residual + out
                for mb in range(2):
                    ps_m = [ps.tile([128, SLICE], f32, tag="psA", bufs=4,
                                    name=f"psm{mb}{i}") for i in range(4)]
                    for k in range(H4 // 128):
                        wt = p2.tile([128, 512], f32r, tag="ws", bufs=5,
                                     name=f"wsm{mb}{k}")
                        nc.sync.dma_start(
                            wt[:], d_wmp[128 * k:128 * (k + 1),
                                         512 * mb:512 * (mb + 1)])
                        rhs = h_sb[k // 4][:, SLICE * (k % 4):
                                           SLICE * (k % 4 + 1)]
                        for m in range(4):
                            nc.tensor.matmul(
                                ps_m[m][:], wt[:, 128 * m:128 * (m + 1)],
                                rhs,
                                start=(k == 0), stop=(k == H4 // 128 - 1))
                    for m in range(4):
                        gm_i = 4 * mb + m
                        x2 = p2.tile([128, SLICE], f32, tag="x2", bufs=3,
                                     name=f"x2_{gm_i}")
                        nc.vector.scalar_tensor_tensor(
                            x2[:], ps_m[m][:], bmp_sb[gm_i][:],
                            x1_sb[gm_i][:],
                            op0=AluOpType.add, op1=AluOpType.add)
                        nc.sync.dma_start(
                            d_out[128 * gm_i:128 * (gm_i + 1), :], x2[:])

    nc.compile()
    return nc


# ------------------------------------------------------------------
# host side
# ------------------------------------------------------------------

_CACHE = {}


def _get_nc(T_b):
    if T_b not in _CACHE:
        _CACHE[T_b] = build(T_b)
    return _CACHE[T_b]


def make_in_maps(x, w_attn, b_attn, w_aproj, b_aproj, ln1_w, ln1_b,
                 ln2_w, ln2_b, w_fc, b_fc, w_mproj, b_mproj):
    Bx, T_b, Hx = x.shape
    QT = T_b // 4
    SLICE = Bx * T_b // NC
    DI = QT // 128
    f = np.float32

    xT = np.ascontiguousarray(
        x.transpose(2, 0, 1).reshape(Hx, Bx * T_b), dtype=f)

    gmask = np.zeros((KCH, 128, NG), f)
    for k in range(KCH):
        for p in range(128):
            gmask[k, p, (128 * k + p) // (Hx // NG)] = 1.0
    gmaskT = np.ascontiguousarray(gmask.transpose(0, 2, 1))

    def msel(ranks):
        m = np.zeros((2, 128, NG), f)
        for r in ranks:
            for g in range(NG):
                row = NG * r + g
                m[row // 128, row % 128, g] = 1.0
        return m

    mb0 = msel(range(4))
    mb1 = msel(range(4, 8))

    cmask = np.zeros((DI, 128, QT), f)
    for di in range(DI):
        kk = np.arange(128)[:, None] + 128 * di
        qq = np.arange(QT)[None, :]
        cmask[di] = np.where(kk <= qq, 1.0, 0.0)

    ones_c = np.ones((128, QT), f)
    id2 = np.eye(2, dtype=f)
    hsel = np.zeros((KCH, 16, 128), f)
    for k in range(KCH):
        for p in range(128):
            hsel[k, (128 * k + p) // 64, p] = 1.0

    in_maps = []
    for c in range(NC):
        h0 = HPC * c
        qcols = np.concatenate(
            [np.arange(64 * (h0 + i), 64 * (h0 + i + 1)) for i in range(HPC)])
        kcols = Hx + qcols
        vcols = 2 * Hx + qcols
        wqk = np.ascontiguousarray(
            np.concatenate([w_attn[:, qcols], w_attn[:, kcols]], axis=1),
            dtype=f)
        wkv = np.ascontiguousarray(
            np.concatenate([w_attn[:, kcols], w_attn[:, vcols]], axis=1),
            dtype=f)
        bqk = np.zeros((2, 256), f)
        bqk[0] = np.concatenate([b_attn[qcols], b_attn[kcols]])
        bkv = np.zeros((2, 256), f)
        bkv[0] = np.concatenate([b_attn[kcols], b_attn[vcols]])
        in_maps.append({
            "xT": xT,
            "x_res": np.ascontiguousarray(xT[:, SLICE * c:SLICE * (c + 1)]),
            "wqk": wqk, "wkv": wkv,
            "bqk": np.ascontiguousarray(bqk, f),
            "bkv": np.ascontiguousarray(bkv, f),
            "w_aproj": np.ascontiguousarray(w_aproj, f),
            "b_aproj": np.asarray(b_aproj, f).reshape(Hx, 1),
            "w_fc": np.ascontiguousarray(w_fc, f),
            "b_fc": np.asarray(b_fc, f).reshape(4 * Hx, 1),
            "w_mproj": np.ascontiguousarray(w_mproj, f),
            "b_mproj": np.asarray(b_mproj, f).reshape(Hx, 1),
            "ln1w": np.asarray(ln1_w, f).reshape(Hx, 1),
            "ln1b": np.asarray(ln1_b, f).reshape(Hx, 1),
            "ln2w": np.asarray(ln2_w, f).reshape(Hx, 1),
            "ln2b": np.asarray(ln2_b, f).reshape(Hx, 1),
            "gmask": gmask, "gmaskT": gmaskT,
            "Mb0": mb0, "Mb1": mb1,
            "Mmy": mb0 if c < 4 else mb1,
            "hsel": hsel,
            "cmaskA": cmask,
            "ones_c": ones_c,
            "id2": id2,
        })
    return in_maps


def kernel(x, w_attn, b_attn, w_aproj, b_aproj, ln1_w, ln1_b, ln2_w, ln2_b,
           w_fc, b_fc, w_mproj, b_mproj, _trace=False):
    from concourse import bass_utils

    x = np.asarray(x, np.float32)
    Bx, T_b, Hx = x.shape
    SLICE = Bx * T_b // NC

    nc = _get_nc(T_b)
    in_maps = make_in_maps(
        x, np.asarray(w_attn, np.float32), np.asarray(b_attn, np.float32),
        np.asarray(w_aproj, np.float32), np.asarray(b_aproj, np.float32),
        np.asarray(ln1_w, np.float32), np.asarray(ln1_b, np.float32),
        np.asarray(ln2_w, np.float32), np.asarray(ln2_b, np.float32),
        np.asarray(w_fc, np.float32), np.asarray(b_fc, np.float32),
        np.asarray(w_mproj, np.float32), np.asarray(b_mproj, np.float32))

    res = bass_utils.run_bass_kernel_spmd(
        nc, in_maps, core_ids=list(range(NC)), trace=_trace)

    xT_out = np.empty((Hx, Bx * T_b), np.float32)
    present = np.empty((2, Bx, NH, T_b, HD), np.float32)
    for c in range(NC):
        r = res.results[c]
        xT_out[:, SLICE * c:SLICE * (c + 1)] = r["x2T"]
        present[:, :, HPC * c:HPC * (c + 1)] = r["present"]
    x_out = np.ascontiguousarray(
        xT_out.reshape(Hx, Bx, T_b).transpose(1, 2, 0))
    kernel.last_exec_time_ns = res.exec_time_ns
    kernel.last_results = res
    return (x_out, present)
